# revision 27
# baseline (speedup 1.0000x reference)
"""CapsuleLayer (dynamic routing) Trainium2 kernel.

Math (see reference): u_hat[b,j,n,o] = sum_i x[b,n,i] W[j,n,i,o]; 3 routing
iterations of softmax-over-j (j=2 -> sigmoid of logit diff) + squash.

Design: shard the n axis (91392) over 8 cores. Everything heavy runs as
K=128-tall PE matmuls over host-packed bf16 layouts (full moving ingest):
  - s-type sums  t[b,(j,o)] = sum_{n,i} y[b,n,i] W[j,n,i,o]
       lhsT = y-slice [128n, 16b], rhs = Ws-slice [128n, 32(j,o)],
       4-way PE column-tiling, PSUM-accumulated.
  - logit pass   z[b,n,i] = sum_{j,o} Vt[b,j,o] W[j,n,i,o]
       lhsT = S4 = I_4 (x) Vt^T [128(il,j,o), 64(g,b)],
       rhs = W4 [128(il,j,o), n] per i-half.
    d[b,n] = sum_i x*z via xz elementwise (DVE) + delta-matmul (PE).
  - w = sigmoid(d) i-replicated (ACT, PSUM-broadcast read); y = w*x (DVE).
A pre-sync AllReduce absorbs cross-core launch skew; two 2KB AllReduces
(after s~0 and t1); final partials are gathered on the host.
"""
import sys

sys.path.insert(0, "/opt/trn_rl_repo")

import numpy as np
import ml_dtypes

BF16 = ml_dtypes.bfloat16
N_CORES = 8
B = 16
NIN = 91392
DI = 8
DO = 16
NC_N = NIN // N_CORES  # 11424
EPS = 1e-7

_CACHE = {}


def _ceil_to(v, m):
    return (v + m - 1) // m * m


def host_prep(x, W, n_cores=N_CORES):
    """Split x [B,N,8] / W [2,N,8,16] into per-core packed bf16 layouts."""
    n_per = x.shape[1] // n_cores
    ncp = _ceil_to(n_per, 1024)
    chunks = ncp // 128
    ngrp = chunks // 8  # XW groups of 8 chunks (1024 n)
    oneD = np.zeros((128, 16), dtype=BF16)
    for i in range(8):
        for b in range(16):
            oneD[i * 16 + b, b] = 1.0
    eye32 = np.eye(32, dtype=np.float32)
    in_maps = []
    for c in range(n_cores):
        xc = np.zeros((B, ncp, DI), dtype=np.float32)
        Wc = np.zeros((2, ncp, DI, DO), dtype=np.float32)
        xc[:, :n_per] = x[:, c * n_per : (c + 1) * n_per]
        Wc[:, :n_per] = W[:, c * n_per : (c + 1) * n_per]
        # xs[n128, (chunk, i, b)] = x[b, n, i]
        xs = (
            xc.reshape(B, chunks, 128, DI)  # b c n i
            .transpose(2, 1, 3, 0)  # n c i b
            .reshape(128, chunks, 128)
        )
        # Ws[n128, (chunk, i, j, o)] = W[j, n, i, o]
        Ws = (
            Wc.reshape(2, chunks, 128, DI, DO)  # j c n i o
            .transpose(2, 1, 3, 0, 4)  # n c i j o
            .reshape(128, chunks, 256)
        )
        # interleave into groups of 8 chunks: [xs(8x128) | ws(8x256)]
        XW = np.empty((128, ngrp, 3072), dtype=BF16)
        xs_g = xs.reshape(128, ngrp, 8 * 128)
        ws_g = Ws.reshape(128, ngrp, 8 * 256)
        XW[:, :, :1024] = xs_g
        XW[:, :, 1024:] = ws_g
        XW = np.ascontiguousarray(XW.reshape(128, ngrp * 3072))
        # W4[(il, j, o), (H, n)] = W[j, n, H*4+il, o]
        W4 = np.ascontiguousarray(
            Wc.reshape(2, ncp, 2, 4, DO)  # j n H il o
            .transpose(3, 0, 4, 2, 1)  # il j o H n
            .reshape(128, 2 * ncp)
        ).astype(BF16)
        # x8[(i, b), n] = x[b, n, i]
        x8 = np.ascontiguousarray(
            xc.transpose(2, 0, 1).reshape(128, ncp)
        ).astype(BF16)
        in_maps.append({"XW": XW, "W4": W4, "x8": x8, "oneD": oneD, "eye32": eye32})
    return in_maps, ncp


def build_kernel(ncp, num_devices=N_CORES):
    from contextlib import ExitStack

    import concourse.bacc as bacc
    import concourse.tile as tile
    from concourse import mybir

    DT = mybir.dt.bfloat16
    F32 = mybir.dt.float32
    AF = mybir.ActivationFunctionType
    chunks = ncp // 128
    zt = ncp // 512
    assert ncp % 1024 == 0
    ngrp = chunks // 8  # XW groups (8 chunks each)
    gz = 4 if zt % 4 == 0 else 1  # z-tiles per W4/x8 load group
    zgrp = zt // gz

    nc = bacc.Bacc(
        "TRN2", target_bir_lowering=False, debug=False, num_devices=num_devices
    )
    xw_in = nc.declare_dram_parameter("XW", [128, ngrp * 3072], DT, isOutput=False)
    w4_in = nc.declare_dram_parameter("W4", [128, 2 * ncp], DT, isOutput=False)
    x8_in = nc.declare_dram_parameter("x8", [128, ncp], DT, isOutput=False)
    oned_in = nc.declare_dram_parameter("oneD", [128, 16], DT, isOutput=False)
    eye_in = nc.declare_dram_parameter("eye32", [32, 32], F32, isOutput=False)
    t2_out = nc.declare_dram_parameter("t2", [16, 32], F32, isOutput=True)
    s0g_out = nc.declare_dram_parameter("s0g", [16, 32], F32, isOutput=True)

    ar_bufs = []
    for k in range(4):
        ar_bufs.append(
            (
                nc.dram_tensor(f"ar_in{k}", [16, 32], F32),
                nc.dram_tensor(f"ar_out{k}", [16, 32], F32, addr_space="Shared"),
            )
        )

    with tile.TileContext(nc) as tc, ExitStack() as ctx:
        park = ctx.enter_context(tc.tile_pool(name="park", bufs=1))
        ps_acc = ctx.enter_context(tc.tile_pool(name="ps_acc", bufs=1, space="PSUM"))
        ps_z = ctx.enter_context(tc.tile_pool(name="ps_z", bufs=4, space="PSUM"))
        ps_d = ctx.enter_context(tc.tile_pool(name="ps_d", bufs=2, space="PSUM"))
        ps_f = ctx.enter_context(tc.tile_pool(name="ps_f", bufs=1, space="PSUM"))
        work = ctx.enter_context(tc.tile_pool(name="work", bufs=6))
        wpool = ctx.enter_context(tc.tile_pool(name="wpool", bufs=3))
        small = ctx.enter_context(tc.tile_pool(name="small", bufs=2))

        # ---- pre-sync: absorb cross-core launch skew under the DMA phase.
        # First a SELF-only reduce (no cross-core wait) to pay the CC-stream
        # first-op warmup cost locally, then a cross-core barrier reduce.
        zt_sb = work.tile([16, 32], F32, tag="zt_sb")
        nc.gpsimd.memset(zt_sb[:], 0.0)
        pre_in, pre_out = ar_bufs[2]
        nc.gpsimd.dma_start(pre_in[:], zt_sb[:])
        nc.gpsimd.collective_compute(
            "AllReduce",
            mybir.AluOpType.add,
            replica_groups=[list(range(num_devices))],
            ins=[pre_in[:]],
            outs=[pre_out[:]],
        )

        # ---- resident input tiles ----
        # Spread the bulk loads over all three DMA-capable engine queues
        # (SP / ACT / POOL) — a single queue tops out well under HBM rate.
        dma_engs = [nc.sync, nc.scalar, nc.gpsimd]
        _dma_rr = [0]

        def load(dst_ap, src_ap):
            dma_engs[_dma_rr[0] % len(dma_engs)].dma_start(dst_ap, src_ap)
            _dma_rr[0] += 1

        # XW first: stage A is paced by these; W4/x8 are only needed after
        # the first AllReduce, so they load in its shadow.
        xw_t = []
        for g in range(ngrp):
            t = park.tile([128, 3072], DT, tag=f"xw{g}")
            load(t[:], xw_in[:, g * 3072 : (g + 1) * 3072])
            xw_t.append(t)
        oneD = park.tile([128, 16], DT, tag="oneD")
        nc.sync.dma_start(oneD[:], oned_in[:])
        eye32 = park.tile([32, 32], F32, tag="eye32")
        nc.sync.dma_start(eye32[:], eye_in[:])
        w4_t = {0: [], 1: []}
        x8_t = []
        for g in range(zgrp):
            c0, c1 = g * gz * 512, (g + 1) * gz * 512
            for h in (0, 1):
                t = park.tile([128, gz * 512], DT, tag=f"w4_{h}_{g}")
                load(t[:], w4_in[:, h * ncp + c0 : h * ncp + c1])
                w4_t[h].append(t)
            t = park.tile([128, gz * 512], DT, tag=f"x8{g}")
            load(t[:], x8_in[:, c0:c1])
            x8_t.append(t)

        def xs_slice(c, w):
            g, lc = c // 8, c % 8
            return xw_t[g][:, lc * 128 : lc * 128 + w]

        def ws_chunk(c):
            g, lc = c // 8, c % 8
            off = 1024 + lc * 256
            return xw_t[g][:, off : off + 256]

        def squash(s_tile, scale):
            """v = squash(scale * s), s_tile [16,32] viewed [16,2,16].

            With u = scale^2*sn and u' = scale*u:
            v = s * u' / (1+u) / sqrt(u+eps)."""
            sq = small.tile([16, 32], F32, tag="sq")
            nc.vector.tensor_mul(sq[:], s_tile[:], s_tile[:])
            sn = small.tile([16, 2], F32, tag="sn")
            nc.vector.tensor_reduce(
                sn[:],
                sq[:].rearrange("p (j o) -> p j o", j=2),
                mybir.AxisListType.X,
                mybir.AluOpType.add,
            )
            up = small.tile([16, 2], F32, tag="up")
            nc.vector.tensor_scalar_mul(up[:], sn[:], scale * scale * scale)
            den = small.tile([16, 2], F32, tag="den")
            nc.vector.tensor_scalar(
                den[:],
                up[:],
                1.0 / scale,
                1.0,
                mybir.AluOpType.mult,
                mybir.AluOpType.add,
            )
            rec = small.tile([16, 2], F32, tag="rec")
            nc.vector.reciprocal(rec[:], den[:])
            epst = small.tile([16, 1], F32, tag="epst")
            nc.vector.memset(epst[:], EPS)
            sr = small.tile([16, 2], F32, tag="sr")
            nc.scalar.activation(
                sr[:], up[:], AF.Sqrt, bias=epst[:], scale=1.0 / scale
            )
            rs = small.tile([16, 2], F32, tag="rs")
            nc.vector.reciprocal(rs[:], sr[:])
            m = small.tile([16, 2], F32, tag="m")
            nc.vector.tensor_mul(m[:], rec[:], rs[:])
            m2 = small.tile([16, 2], F32, tag="m2")
            nc.vector.tensor_mul(m2[:], up[:], m[:])
            v = small.tile([16, 32], F32, tag=f"v_{scale}_{nc.next_id()}")
            nc.vector.tensor_mul(
                v[:].rearrange("p (j o) -> p j o", j=2),
                s_tile[:].rearrange("p (j o) -> p j o", j=2),
                m2[:].unsqueeze(2).broadcast_to([16, 2, 16]),
            )
            return v

        def all_reduce(src_sb, idx):
            """SBUF [16,32] partial -> SBUF tile of the global sum."""
            a_in, a_out = ar_bufs[idx]
            nc.scalar.dma_start(a_in[:], src_sb[:])
            nc.gpsimd.collective_compute(
                "AllReduce",
                mybir.AluOpType.add,
                replica_groups=[list(range(num_devices))],
                ins=[a_in[:]],
                outs=[a_out[:]],
            )
            g = small.tile([16, 32], F32, tag=f"arg{idx}")
            nc.gpsimd.dma_start(g[:], a_out[:])
            return g

        def fold_diag(st_ps, tag):
            """Extract+sum the 8 diagonal 16x32 blocks of a [128,256] PSUM acc.

            PSUM reads need 32-aligned partition bases: sum the four aligned
            [32,64] quadrants into SBUF, then fold the two 16x32 diagonals
            with two identity matmuls (a base-16 DVE read is illegal)."""
            prev = small.tile([32, 64], F32, tag=f"qs0_{tag}")
            nc.vector.tensor_copy(prev[:], st_ps[0:32, 0:64])
            for q in (1, 2, 3):
                nxt = small.tile([32, 64], F32, tag=f"qs{q}_{tag}")
                nc.vector.tensor_add(
                    nxt[:],
                    prev[:],
                    st_ps[32 * q : 32 * q + 32, 64 * q : 64 * q + 64],
                )
                prev = nxt
            fold_ps = ps_f.tile([16, 32], F32, tag="fold")
            nc.tensor.matmul(
                fold_ps[:], eye32[:, 0:16], prev[:, 0:32], start=True, stop=False
            )
            nc.tensor.matmul(
                fold_ps[:], eye32[:, 16:32], prev[:, 32:64], start=False, stop=True
            )
            acc = small.tile([16, 32], F32, tag=f"acc_{tag}")
            nc.vector.tensor_copy(acc[:], fold_ps[:])
            return acc

        def s_sweep(lhs_for_chunk, tag):
            """t[b,(j,o)] = sum_{c,i} y[n,(i,b)]^T @ Ws[n,(i,jo)].

            One [128,128]^T @ [128,256] matmul per 128-n chunk, PSUM-accumulated
            over all chunks. The 8 diagonal 16x32 blocks of the [128,256] result
            are the per-i partial sums (off-diagonal i!=i' cross terms accumulate
            harmlessly and are never read)."""
            st_ps = ps_acc.tile([128, 256], F32, tag="stacc")
            for c in range(chunks):
                nc.tensor.matmul(
                    st_ps[:],
                    lhs_for_chunk(c),
                    ws_chunk(c),
                    start=(c == 0),
                    stop=(c == chunks - 1),
                )
            return fold_diag(st_ps, tag)

        # ---- stage A: st0[b,(j,o)] = sum_{n,i} x W ----
        st0_sb = s_sweep(lambda c: xs_slice(c, 128), "a")
        st0g = all_reduce(st0_sb, 0)
        nc.sync.dma_start(s0g_out[:], st0g[:])
        v0 = squash(st0g, 0.5)

        def routing_pass(vacc, it):
            """Given accumulated v [16,32], compute t[b,(j,o)] partial (SBUF)."""
            # Vt transposed + sign: vT[(j,o), b] = +/- vacc[b, (j,o)]
            vt_in = work.tile([32, 32], F32, tag="vt_in")
            nc.vector.memset(vt_in[:], 0.0)
            nc.vector.tensor_copy(vt_in[0:16, 0:16], vacc[:, 0:16])
            nc.scalar.mul(vt_in[0:16, 16:32], vacc[:, 16:32], -1.0)
            vT = work.tile([32, 32], F32, tag="vT")
            nc.vector.transpose(vT[:], vt_in[:])
            # S4 = I_4 (x) vT : [128 (il,j,o), 64 (g,b)]
            s4 = work.tile([128, 64], DT, tag="s4")
            nc.vector.memset(s4[:], 0.0)
            for gg in range(4):
                nc.scalar.copy(
                    s4[gg * 32 : gg * 32 + 32, gg * 16 : gg * 16 + 16],
                    vT[0:32, 0:16],
                )
            # Software-pipelined per-tile loop. PE program order per tile:
            #   z-pair(t) -> sweep MMs of tile t-1 -> d-MMs(t)
            # so the PE has dense work while DVE computes xz(t); this keeps
            # the HAM activity monitor at K=8/8 (cold PE ran pass MMs at
            # 1.2 GHz in the unpipelined version).
            st_ps = ps_acc.tile([128, 256], F32, tag="stacc")

            def sweep_tile(t, y4):
                for cc in range(4):
                    c = t * 4 + cc
                    nc.tensor.matmul(
                        st_ps[:],
                        y4[:, cc * 128 : cc * 128 + 128],
                        ws_chunk(c),
                        start=(c == 0),
                        stop=(c == chunks - 1),
                        skip_group_check=True,
                    )

            ys = {}
            for t in range(zt):
                zg, off = t // gz, (t % gz) * 512
                z_ps = ps_z.tile([128, 512], F32, tag="z")
                # high_priority keeps the H-pair adjacent in the PE stream so
                # the two column-strip matmuls overlap (a full-width sweep MM
                # between them blocks the second strip's LDWEIGHTS)
                with tc.high_priority():
                    for H in (0, 1):
                        nc.tensor.matmul(
                            z_ps[H * 64 : H * 64 + 64, :],
                            s4[:, 0:64],
                            w4_t[H][zg][:, off : off + 512],
                            start=True,
                            stop=True,
                            tile_position=(0, H * 64),
                            skip_group_check=True,
                        )
                if t >= 3:
                    sweep_tile(t - 3, ys.pop(t - 3))
                xz = work.tile([128, 512], DT, tag="xz")
                nc.vector.tensor_mul(xz[:], z_ps[:], x8_t[zg][:, off : off + 512])
                # d[n,(csub,b)] for the whole 512-n tile in one PSUM region
                d_ps = ps_d.tile([128, 64], F32, tag="d")
                for k4 in range(4):
                    nc.tensor.matmul(
                        d_ps[:, k4 * 16 : k4 * 16 + 16],
                        xz[:, k4 * 128 : k4 * 128 + 128],
                        oneD[:],
                        start=True,
                        stop=True,
                    )
                # sigmoid without i-replication: w_sm[n,(csub,b)]; the y-muls
                # broadcast it over i via the read AP instead
                w_sm = wpool.tile([128, 64], DT, tag="w")
                nc.scalar.activation(w_sm[:], d_ps[:], AF.Sigmoid)
                # y = w * x split DVE (1 chunk) / GpSimd (3 chunks) to balance
                y4 = work.tile([128, 512], DT, tag="y")
                nc.vector.tensor_mul(
                    y4[:, 0:128].rearrange("p (i b) -> p i b", i=8),
                    xs_slice(4 * t, 128).rearrange("p (i b) -> p i b", i=8),
                    w_sm[:, 0:16].unsqueeze(1).broadcast_to([128, 8, 16]),
                )
                nc.gpsimd.tensor_mul(
                    y4[:, 128:512].rearrange("p (c i b) -> p c i b", c=3, i=8),
                    xs_slice(4 * t + 1, 384).rearrange(
                        "p (c i b) -> p c i b", c=3, i=8
                    ),
                    w_sm[:, 16:64]
                    .rearrange("p (c b) -> p c b", c=3)
                    .unsqueeze(2)
                    .broadcast_to([128, 3, 8, 16]),
                )
                ys[t] = y4
            for tt in (zt - 3, zt - 2, zt - 1):
                sweep_tile(tt, ys.pop(tt))
            return fold_diag(st_ps, f"i{it}")

        # ---- iteration 1 ----
        t1_sb = routing_pass(v0, 1)
        t1g = all_reduce(t1_sb, 1)
        s1 = small.tile([16, 32], F32, tag="s1")
        nc.vector.tensor_copy(s1[:, 0:16], t1g[:, 0:16])
        nc.vector.tensor_sub(s1[:, 16:32], st0g[:, 16:32], t1g[:, 16:32])
        v1 = squash(s1, 1.0)
        vacc2 = small.tile([16, 32], F32, tag="vacc2")
        nc.vector.tensor_add(vacc2[:], v0[:], v1[:])

        # ---- iteration 2 (partials out; host combines) ----
        t2_sb = routing_pass(vacc2, 2)
        nc.sync.dma_start(t2_out[:], t2_sb[:])

    nc.compile()
    return nc


def _squash_np(s):
    sn = np.sum(s * s, axis=-1, keepdims=True)
    return sn / (1.0 + sn) / np.sqrt(sn + EPS) * s


def finish_host(results):
    """Combine per-core (t2, s0g) partials into v2 [16,2,16]."""
    t2 = sum(np.asarray(r["t2"], dtype=np.float64) for r in results)
    s0g = np.asarray(results[0]["s0g"], dtype=np.float64)
    s2 = np.empty((16, 2, 16), dtype=np.float64)
    s2[:, 0, :] = t2[:, 0:16]
    s2[:, 1, :] = s0g[:, 16:32] - t2[:, 16:32]
    return _squash_np(s2).astype(np.float32)


def run(x, W, **spmd_kwargs):
    from concourse.bass_utils import run_bass_kernel_spmd

    x = np.asarray(x, dtype=np.float32)
    W = np.asarray(W, dtype=np.float32)
    in_maps, ncp = host_prep(x, W)
    key = ("nc", ncp)
    if key not in _CACHE:
        _CACHE[key] = build_kernel(ncp)
    nc = _CACHE[key]
    res = run_bass_kernel_spmd(nc, in_maps, list(range(N_CORES)), **spmd_kwargs)
    return finish_host(res.results), res


def kernel(x, W):
    return run(x, W)[0]



# revision 29
# speedup vs baseline: 1.1277x; 1.1277x over previous
"""CapsuleLayer (dynamic routing) Trainium2 kernel.

Math (see reference): u_hat[b,j,n,o] = sum_i x[b,n,i] W[j,n,i,o]; 3 routing
iterations of softmax-over-j (j=2 -> sigmoid of logit diff) + squash.

Design: shard the n axis (91392) over 8 cores. Everything heavy runs as
K=128-tall PE matmuls over host-packed bf16 layouts (full moving ingest):
  - s-type sums  t[b,(j,o)] = sum_{n,i} y[b,n,i] W[j,n,i,o]
       lhsT = y-slice [128n, 16b], rhs = Ws-slice [128n, 32(j,o)],
       4-way PE column-tiling, PSUM-accumulated.
  - logit pass   z[b,n,i] = sum_{j,o} Vt[b,j,o] W[j,n,i,o]
       lhsT = S4 = I_4 (x) Vt^T [128(il,j,o), 64(g,b)],
       rhs = W4 [128(il,j,o), n] per i-half.
    d[b,n] = sum_i x*z via xz elementwise (DVE) + delta-matmul (PE).
  - w = sigmoid(d) i-replicated (ACT, PSUM-broadcast read); y = w*x (DVE).
A pre-sync AllReduce absorbs cross-core launch skew; two 2KB AllReduces
(after s~0 and t1); final partials are gathered on the host.
"""
import sys

sys.path.insert(0, "/opt/trn_rl_repo")

import numpy as np
import ml_dtypes

BF16 = ml_dtypes.bfloat16
N_CORES = 8
B = 16
NIN = 91392
DI = 8
DO = 16
NC_N = NIN // N_CORES  # 11424
EPS = 1e-7

_CACHE = {}


def _ceil_to(v, m):
    return (v + m - 1) // m * m


def host_prep(x, W, n_cores=N_CORES):
    """Split x [B,N,8] / W [2,N,8,16] into per-core packed bf16 layouts."""
    n_per = x.shape[1] // n_cores
    ncp = _ceil_to(n_per, 1024)
    chunks = ncp // 128
    ngrp = chunks // 8  # XW groups of 8 chunks (1024 n)
    oneD = np.zeros((128, 16), dtype=BF16)
    for i in range(8):
        for b in range(16):
            oneD[i * 16 + b, b] = 1.0
    eye32 = np.eye(32, dtype=np.float32)
    in_maps = []
    for c in range(n_cores):
        xc = np.zeros((B, ncp, DI), dtype=np.float32)
        Wc = np.zeros((2, ncp, DI, DO), dtype=np.float32)
        xc[:, :n_per] = x[:, c * n_per : (c + 1) * n_per]
        Wc[:, :n_per] = W[:, c * n_per : (c + 1) * n_per]
        # xs[n128, (chunk, i, b)] = x[b, n, i]
        xs = (
            xc.reshape(B, chunks, 128, DI)  # b c n i
            .transpose(2, 1, 3, 0)  # n c i b
            .reshape(128, chunks, 128)
        )
        # Ws[n128, (chunk, i, j, o)] = W[j, n, i, o]
        Ws = (
            Wc.reshape(2, chunks, 128, DI, DO)  # j c n i o
            .transpose(2, 1, 3, 0, 4)  # n c i j o
            .reshape(128, chunks, 256)
        )
        # interleave into groups of 8 chunks: [xs(8x128) | ws(8x256)]
        XW = np.empty((128, ngrp, 3072), dtype=BF16)
        xs_g = xs.reshape(128, ngrp, 8 * 128)
        ws_g = Ws.reshape(128, ngrp, 8 * 256)
        XW[:, :, :1024] = xs_g
        XW[:, :, 1024:] = ws_g
        XW = np.ascontiguousarray(XW.reshape(128, ngrp * 3072))
        # W4[(il, j, o), (H, n)] = W[j, n, H*4+il, o]
        W4 = np.ascontiguousarray(
            Wc.reshape(2, ncp, 2, 4, DO)  # j n H il o
            .transpose(3, 0, 4, 2, 1)  # il j o H n
            .reshape(128, 2 * ncp)
        ).astype(BF16)
        # x8[(i, b), n] = x[b, n, i]
        x8 = np.ascontiguousarray(
            xc.transpose(2, 0, 1).reshape(128, ncp)
        ).astype(BF16)
        in_maps.append({"XW": XW, "W4": W4, "x8": x8, "oneD": oneD, "eye32": eye32})
    return in_maps, ncp


def build_kernel(ncp, num_devices=N_CORES):
    from contextlib import ExitStack

    import concourse.bacc as bacc
    import concourse.tile as tile
    from concourse import mybir

    DT = mybir.dt.bfloat16
    F32 = mybir.dt.float32
    AF = mybir.ActivationFunctionType
    chunks = ncp // 128
    zt = ncp // 512
    assert ncp % 1024 == 0
    ngrp = chunks // 8  # XW groups (8 chunks each)
    gz = 4 if zt % 4 == 0 else 1  # z-tiles per W4/x8 load group
    zgrp = zt // gz

    nc = bacc.Bacc(
        "TRN2", target_bir_lowering=False, debug=False, num_devices=num_devices
    )
    xw_in = nc.declare_dram_parameter("XW", [128, ngrp * 3072], DT, isOutput=False)
    w4_in = nc.declare_dram_parameter("W4", [128, 2 * ncp], DT, isOutput=False)
    x8_in = nc.declare_dram_parameter("x8", [128, ncp], DT, isOutput=False)
    oned_in = nc.declare_dram_parameter("oneD", [128, 16], DT, isOutput=False)
    eye_in = nc.declare_dram_parameter("eye32", [32, 32], F32, isOutput=False)
    t2_out = nc.declare_dram_parameter("t2", [32, 64], F32, isOutput=True)
    s0g_out = nc.declare_dram_parameter("s0g", [16, 32], F32, isOutput=True)

    ar_bufs = []
    for k in range(4):
        ar_bufs.append(
            (
                nc.dram_tensor(f"ar_in{k}", [16, 32], F32),
                nc.dram_tensor(f"ar_out{k}", [16, 32], F32, addr_space="Shared"),
            )
        )

    with tile.TileContext(nc) as tc, ExitStack() as ctx:
        park = ctx.enter_context(tc.tile_pool(name="park", bufs=1))
        ps_acc = ctx.enter_context(tc.tile_pool(name="ps_acc", bufs=1, space="PSUM"))
        ps_z = ctx.enter_context(tc.tile_pool(name="ps_z", bufs=4, space="PSUM"))
        ps_d = ctx.enter_context(tc.tile_pool(name="ps_d", bufs=2, space="PSUM"))
        ps_f = ctx.enter_context(tc.tile_pool(name="ps_f", bufs=1, space="PSUM"))
        work = ctx.enter_context(tc.tile_pool(name="work", bufs=6))
        wpool = ctx.enter_context(tc.tile_pool(name="wpool", bufs=3))
        small = ctx.enter_context(tc.tile_pool(name="small", bufs=2))

        # ---- pre-sync: absorb cross-core launch skew under the DMA phase.
        # First a SELF-only reduce (no cross-core wait) to pay the CC-stream
        # first-op warmup cost locally, then a cross-core barrier reduce.
        zt_sb = work.tile([16, 32], F32, tag="zt_sb")
        nc.gpsimd.memset(zt_sb[:], 0.0)
        pre_in, pre_out = ar_bufs[2]
        nc.gpsimd.dma_start(pre_in[:], zt_sb[:])
        nc.gpsimd.collective_compute(
            "AllReduce",
            mybir.AluOpType.add,
            replica_groups=[list(range(num_devices))],
            ins=[pre_in[:]],
            outs=[pre_out[:]],
        )

        # ---- resident input tiles ----
        # Spread the bulk loads over all three DMA-capable engine queues
        # (SP / ACT / POOL) — a single queue tops out well under HBM rate.
        dma_engs = [nc.sync, nc.scalar, nc.gpsimd]
        _dma_rr = [0]

        def load(dst_ap, src_ap):
            dma_engs[_dma_rr[0] % len(dma_engs)].dma_start(dst_ap, src_ap)
            _dma_rr[0] += 1

        # XW first: stage A is paced by these; W4/x8 are only needed after
        # the first AllReduce, so they load in its shadow.
        xw_t = []
        for g in range(ngrp):
            t = park.tile([128, 3072], DT, tag=f"xw{g}")
            load(t[:], xw_in[:, g * 3072 : (g + 1) * 3072])
            xw_t.append(t)
        oneD = park.tile([128, 16], DT, tag="oneD")
        nc.sync.dma_start(oneD[:], oned_in[:])
        eye32 = park.tile([32, 32], F32, tag="eye32")
        nc.sync.dma_start(eye32[:], eye_in[:])
        w4_t = {0: [], 1: []}
        x8_t = []
        for g in range(zgrp):
            c0, c1 = g * gz * 512, (g + 1) * gz * 512
            for h in (0, 1):
                t = park.tile([128, gz * 512], DT, tag=f"w4_{h}_{g}")
                load(t[:], w4_in[:, h * ncp + c0 : h * ncp + c1])
                w4_t[h].append(t)
            t = park.tile([128, gz * 512], DT, tag=f"x8{g}")
            load(t[:], x8_in[:, c0:c1])
            x8_t.append(t)

        def xs_slice(c, w):
            g, lc = c // 8, c % 8
            return xw_t[g][:, lc * 128 : lc * 128 + w]

        def ws_chunk(c):
            g, lc = c // 8, c % 8
            off = 1024 + lc * 256
            return xw_t[g][:, off : off + 256]

        def squash(s_tile, scale):
            """v = squash(scale * s), s_tile [16,32] viewed [16,2,16].

            With u = scale^2*sn and u' = scale*u:
            v = s * u' / (1+u) / sqrt(u+eps)."""
            sq = small.tile([16, 32], F32, tag="sq")
            nc.vector.tensor_mul(sq[:], s_tile[:], s_tile[:])
            sn = small.tile([16, 2], F32, tag="sn")
            nc.vector.tensor_reduce(
                sn[:],
                sq[:].rearrange("p (j o) -> p j o", j=2),
                mybir.AxisListType.X,
                mybir.AluOpType.add,
            )
            up = small.tile([16, 2], F32, tag="up")
            nc.vector.tensor_scalar_mul(up[:], sn[:], scale * scale * scale)
            den = small.tile([16, 2], F32, tag="den")
            nc.vector.tensor_scalar(
                den[:],
                up[:],
                1.0 / scale,
                1.0,
                mybir.AluOpType.mult,
                mybir.AluOpType.add,
            )
            rec = small.tile([16, 2], F32, tag="rec")
            nc.vector.reciprocal(rec[:], den[:])
            epst = small.tile([16, 1], F32, tag="epst")
            nc.vector.memset(epst[:], EPS)
            sr = small.tile([16, 2], F32, tag="sr")
            nc.scalar.activation(
                sr[:], up[:], AF.Sqrt, bias=epst[:], scale=1.0 / scale
            )
            rs = small.tile([16, 2], F32, tag="rs")
            nc.vector.reciprocal(rs[:], sr[:])
            m = small.tile([16, 2], F32, tag="m")
            nc.vector.tensor_mul(m[:], rec[:], rs[:])
            m2 = small.tile([16, 2], F32, tag="m2")
            nc.vector.tensor_mul(m2[:], up[:], m[:])
            v = small.tile([16, 32], F32, tag=f"v_{scale}_{nc.next_id()}")
            nc.vector.tensor_mul(
                v[:].rearrange("p (j o) -> p j o", j=2),
                s_tile[:].rearrange("p (j o) -> p j o", j=2),
                m2[:].unsqueeze(2).broadcast_to([16, 2, 16]),
            )
            return v

        def all_reduce(src_sb, idx):
            """SBUF [16,32] partial -> SBUF tile of the global sum."""
            a_in, a_out = ar_bufs[idx]
            nc.scalar.dma_start(a_in[:], src_sb[:])
            nc.gpsimd.collective_compute(
                "AllReduce",
                mybir.AluOpType.add,
                replica_groups=[list(range(num_devices))],
                ins=[a_in[:]],
                outs=[a_out[:]],
            )
            g = small.tile([16, 32], F32, tag=f"arg{idx}")
            nc.gpsimd.dma_start(g[:], a_out[:])
            return g

        def fold_diag(st_ps, tag, fold=True):
            """Extract+sum the 8 diagonal 16x32 blocks of a [128,256] PSUM acc.

            PSUM reads need 32-aligned partition bases: tree-sum the four
            aligned [32,64] quadrants into SBUF, then fold the two 16x32
            diagonals with two identity matmuls (a base-16 DVE read is
            illegal). With fold=False, return the [32,64] quadrant sum (the
            host folds)."""
            prev = small.tile([32, 64], F32, tag=f"qs0_{tag}")
            nc.vector.tensor_copy(prev[:], st_ps[0:32, 0:64])
            for q in (1, 2, 3):
                nxt = small.tile([32, 64], F32, tag=f"qs{q}_{tag}")
                nc.vector.tensor_add(
                    nxt[:],
                    prev[:],
                    st_ps[32 * q : 32 * q + 32, 64 * q : 64 * q + 64],
                )
                prev = nxt
            if not fold:
                return prev
            fold_ps = ps_f.tile([16, 32], F32, tag="fold")
            nc.tensor.matmul(
                fold_ps[:], eye32[:, 0:16], prev[:, 0:32], start=True, stop=False
            )
            nc.tensor.matmul(
                fold_ps[:], eye32[:, 16:32], prev[:, 32:64], start=False, stop=True
            )
            acc = small.tile([16, 32], F32, tag=f"acc_{tag}")
            nc.vector.tensor_copy(acc[:], fold_ps[:])
            return acc

        def s_sweep(lhs_for_chunk, tag):
            """t[b,(j,o)] = sum_{c,i} y[n,(i,b)]^T @ Ws[n,(i,jo)].

            One [128,128]^T @ [128,256] matmul per 128-n chunk, PSUM-accumulated
            over all chunks. The 8 diagonal 16x32 blocks of the [128,256] result
            are the per-i partial sums (off-diagonal i!=i' cross terms accumulate
            harmlessly and are never read)."""
            st_ps = ps_acc.tile([128, 256], F32, tag="stacc")
            for c in range(chunks):
                nc.tensor.matmul(
                    st_ps[:],
                    lhs_for_chunk(c),
                    ws_chunk(c),
                    start=(c == 0),
                    stop=(c == chunks - 1),
                )
            return fold_diag(st_ps, tag)

        # ---- stage A: st0[b,(j,o)] = sum_{n,i} x W ----
        st0_sb = s_sweep(lambda c: xs_slice(c, 128), "a")
        st0g = all_reduce(st0_sb, 0)
        nc.sync.dma_start(s0g_out[:], st0g[:])
        v0 = squash(st0g, 0.5)

        def routing_pass(vacc, it):
            """Given accumulated v [16,32], compute t[b,(j,o)] partial (SBUF)."""
            # Vt transposed + sign: vT[(j,o), b] = +/- vacc[b, (j,o)]
            vt_in = work.tile([32, 32], F32, tag="vt_in")
            nc.vector.memset(vt_in[:], 0.0)
            nc.vector.tensor_copy(vt_in[0:16, 0:16], vacc[:, 0:16])
            nc.scalar.mul(vt_in[0:16, 16:32], vacc[:, 16:32], -1.0)
            vT = work.tile([32, 32], F32, tag="vT")
            nc.vector.transpose(vT[:], vt_in[:])
            # S4 = I_4 (x) vT : [128 (il,j,o), 64 (g,b)]
            s4 = work.tile([128, 64], DT, tag="s4")
            nc.vector.memset(s4[:], 0.0)
            for gg in range(4):
                nc.scalar.copy(
                    s4[gg * 32 : gg * 32 + 32, gg * 16 : gg * 16 + 16],
                    vT[0:32, 0:16],
                )
            # Software-pipelined per-tile loop. PE program order per tile:
            #   z-pair(t) -> sweep MMs of tile t-1 -> d-MMs(t)
            # so the PE has dense work while DVE computes xz(t); this keeps
            # the HAM activity monitor at K=8/8 (cold PE ran pass MMs at
            # 1.2 GHz in the unpipelined version).
            st_ps = ps_acc.tile([128, 256], F32, tag="stacc")

            def sweep_tile(t, y4):
                for cc in range(4):
                    c = t * 4 + cc
                    nc.tensor.matmul(
                        st_ps[:],
                        y4[:, cc * 128 : cc * 128 + 128],
                        ws_chunk(c),
                        start=(c == 0),
                        stop=(c == chunks - 1),
                        skip_group_check=True,
                    )

            ys = {}
            for t in range(zt):
                zg, off = t // gz, (t % gz) * 512
                z_ps = ps_z.tile([128, 512], F32, tag="z")
                # high_priority keeps the H-pair adjacent in the PE stream so
                # the two column-strip matmuls overlap (a full-width sweep MM
                # between them blocks the second strip's LDWEIGHTS)
                with tc.high_priority():
                    for H in (0, 1):
                        nc.tensor.matmul(
                            z_ps[H * 64 : H * 64 + 64, :],
                            s4[:, 0:64],
                            w4_t[H][zg][:, off : off + 512],
                            start=True,
                            stop=True,
                            tile_position=(0, H * 64),
                            skip_group_check=True,
                        )
                if t >= 3:
                    sweep_tile(t - 3, ys.pop(t - 3))
                xz = work.tile([128, 512], DT, tag="xz")
                nc.vector.tensor_mul(xz[:], z_ps[:], x8_t[zg][:, off : off + 512])
                # d[n,(csub,b)] for the whole 512-n tile in one PSUM region
                d_ps = ps_d.tile([128, 64], F32, tag="d")
                for k4 in range(4):
                    nc.tensor.matmul(
                        d_ps[:, k4 * 16 : k4 * 16 + 16],
                        xz[:, k4 * 128 : k4 * 128 + 128],
                        oneD[:],
                        start=True,
                        stop=True,
                    )
                # sigmoid without i-replication: w_sm[n,(csub,b)]; the y-muls
                # broadcast it over i via the read AP instead
                w_sm = wpool.tile([128, 64], DT, tag="w")
                nc.scalar.activation(w_sm[:], d_ps[:], AF.Sigmoid)
                # y = w * x split DVE (1 chunk) / GpSimd (3 chunks) to balance
                y4 = work.tile([128, 512], DT, tag="y")
                nc.vector.tensor_mul(
                    y4[:, 0:256].rearrange("p (c i b) -> p c i b", c=2, i=8),
                    xs_slice(4 * t, 256).rearrange("p (c i b) -> p c i b", c=2, i=8),
                    w_sm[:, 0:32]
                    .rearrange("p (c b) -> p c b", c=2)
                    .unsqueeze(2)
                    .broadcast_to([128, 2, 8, 16]),
                )
                nc.gpsimd.tensor_mul(
                    y4[:, 256:512].rearrange("p (c i b) -> p c i b", c=2, i=8),
                    xs_slice(4 * t + 2, 256).rearrange(
                        "p (c i b) -> p c i b", c=2, i=8
                    ),
                    w_sm[:, 32:64]
                    .rearrange("p (c b) -> p c b", c=2)
                    .unsqueeze(2)
                    .broadcast_to([128, 2, 8, 16]),
                )
                ys[t] = y4
            for tt in (zt - 3, zt - 2, zt - 1):
                sweep_tile(tt, ys.pop(tt))
            return fold_diag(st_ps, f"i{it}", fold=(it != 2))

        # ---- iteration 1 ----
        t1_sb = routing_pass(v0, 1)
        t1g = all_reduce(t1_sb, 1)
        s1 = small.tile([16, 32], F32, tag="s1")
        nc.vector.tensor_copy(s1[:, 0:16], t1g[:, 0:16])
        nc.vector.tensor_sub(s1[:, 16:32], st0g[:, 16:32], t1g[:, 16:32])
        v1 = squash(s1, 1.0)
        vacc2 = small.tile([16, 32], F32, tag="vacc2")
        nc.vector.tensor_add(vacc2[:], v0[:], v1[:])

        # ---- iteration 2 (partials out; host combines) ----
        t2_sb = routing_pass(vacc2, 2)
        nc.sync.dma_start(t2_out[:], t2_sb[:])

    nc.compile()
    return nc


def _squash_np(s):
    sn = np.sum(s * s, axis=-1, keepdims=True)
    return sn / (1.0 + sn) / np.sqrt(sn + EPS) * s


def finish_host(results):
    """Combine per-core (t2, s0g) partials into v2 [16,2,16]."""
    q = sum(np.asarray(r["t2"], dtype=np.float64) for r in results)
    t2 = q[0:16, 0:32] + q[16:32, 32:64]
    s0g = np.asarray(results[0]["s0g"], dtype=np.float64)
    s2 = np.empty((16, 2, 16), dtype=np.float64)
    s2[:, 0, :] = t2[:, 0:16]
    s2[:, 1, :] = s0g[:, 16:32] - t2[:, 16:32]
    return _squash_np(s2).astype(np.float32)


def run(x, W, **spmd_kwargs):
    from concourse.bass_utils import run_bass_kernel_spmd

    x = np.asarray(x, dtype=np.float32)
    W = np.asarray(W, dtype=np.float32)
    in_maps, ncp = host_prep(x, W)
    key = ("nc", ncp)
    if key not in _CACHE:
        _CACHE[key] = build_kernel(ncp)
    nc = _CACHE[key]
    res = run_bass_kernel_spmd(nc, in_maps, list(range(N_CORES)), **spmd_kwargs)
    return finish_host(res.results), res


def kernel(x, W):
    return run(x, W)[0]

